# revision 1
# baseline (speedup 1.0000x reference)
"""Kernel builder for causal self-attention (RoPE + parameter-free RMSNorm on Q/K).

Sharding: 8 cores = 4 batch x 2 head-groups (8 heads each). Each core computes
its batch element's attention for its 8 heads plus the partial output
projection; host sums the two head-group partials per batch element.

Per-core device layout (D=64, 8 heads):
  Q^T / K^T stored as [128, 4, T]: col j = 128*cc + p,
     cc = 2*(h//4) + half, p = 32*(h%4) + r,  (d = 32*half + r)
  V stored with a ones column per head: [128, T//128, 8*65]; the ones column
  makes the PV matmul also accumulate the softmax denominator (row 64).
  Scores computed transposed: S^T[tk, tq] per head via K=32 row-tiled matmuls;
  softmax runs without max-subtraction (RMS-normed q,k bound |s| <= 8);
  the denominator division folds in before the output projection.
"""

import sys

import numpy as np

for _p in ("/opt/trn_rl_repo",):
    if _p not in sys.path:
        sys.path.insert(0, _p)

import concourse.bass as bass
import concourse.mybir as mybir
import concourse.tile as tile
from concourse import bacc

F32 = mybir.dt.float32
F32R = mybir.dt.float32r
AX = mybir.AluOpType
ACTF = mybir.ActivationFunctionType

D = 64
NH = 8          # heads per core
CH = NH * D     # 512 head channels per core
EPS = float(np.finfo(np.float32).eps)


def qk_col_perm():
    """perm[j] = plain column (64*h + d) stored at device column j."""
    perm = np.zeros(CH, dtype=np.int64)
    for h in range(NH):
        for half in range(2):
            for r in range(32):
                j = 128 * (2 * (h // 4) + half) + 32 * (h % 4) + r
                perm[j] = 64 * h + 32 * half + r
    return perm


def make_consts(T):
    """Host-side constant tensors fed as kernel inputs."""
    cs_d = D // 2
    inv_freq = 1.0 / (10000.0 ** (np.arange(cs_d, dtype=np.float64) / cs_d))
    freqs = np.outer(np.arange(T, dtype=np.float64), inv_freq)  # [T, 32]
    cosT = np.cos(freqs).astype(np.float32).T  # [32, T]
    sinT = np.sin(freqs).astype(np.float32).T
    COS = np.tile(cosT, (4, 1))  # [128, T]
    SIN = np.tile(sinT, (4, 1))
    # Boundary mask strip [128, 128]: MASK[p, j] = (p <= j)
    p = np.arange(128)[:, None]
    j = np.arange(128)[None, :]
    MASK = (p <= j).astype(np.float32)
    # SEL for ssq reduction: SEL_g[p, m] = 1 if m == 4*g + p//32  ([128, 8])
    SELA = np.zeros((128, 8), dtype=np.float32)
    SELB = np.zeros((128, 8), dtype=np.float32)
    for pp in range(128):
        SELA[pp, pp // 32] = 1.0
        SELB[pp, 4 + pp // 32] = 1.0
    SELTA = SELA.T.copy()
    SELTB = SELB.T.copy()
    ONESF = np.ones((128, 128), dtype=np.float32)
    return dict(COS=COS, SIN=SIN, MASK=MASK, SELA=SELA, SELB=SELB,
                SELTA=SELTA, SELTB=SELTB, ONESF=ONESF)


def make_core_inputs(x_b, Wq_s, Wk_s, Wv_s, Wo_s, consts):
    """x_b [T, CIN]; W*_s are this core's shards: Wq/Wk/Wv [CIN, 512] (plain
    column order 64h+d), Wo_s [512, COUT]. Returns the kernel input map."""
    perm = qk_col_perm()
    return dict(
        xT=np.ascontiguousarray(x_b.T),
        Wq=np.ascontiguousarray(Wq_s[:, perm]),
        Wk=np.ascontiguousarray(Wk_s[:, perm]),
        Wv=np.ascontiguousarray(Wv_s),
        Wo=np.ascontiguousarray(Wo_s),
        **{k: np.ascontiguousarray(v) for k, v in consts.items()},
    )


def build_nc(T, CIN, COUT):
    """Build the Bass program. T seq len, CIN input channels, COUT out channels."""
    assert T % 512 == 0 and CIN % 128 == 0 and COUT % 512 == 0
    KC = CIN // 128        # c_in chunks
    NTB = T // 512         # projection t-blocks == tq blocks
    NQ = T // 512
    NKC = T // 128         # tk chunks
    NCO = COUT // 512      # out-proj column halves

    nc = bacc.Bacc()

    xT = nc.dram_tensor("xT", [CIN, T], F32, kind="ExternalInput")
    Wq = nc.dram_tensor("Wq", [CIN, CH], F32, kind="ExternalInput")
    Wk = nc.dram_tensor("Wk", [CIN, CH], F32, kind="ExternalInput")
    Wv = nc.dram_tensor("Wv", [CIN, CH], F32, kind="ExternalInput")
    Wo = nc.dram_tensor("Wo", [CH, COUT], F32, kind="ExternalInput")
    COS = nc.dram_tensor("COS", [128, T], F32, kind="ExternalInput")
    SIN = nc.dram_tensor("SIN", [128, T], F32, kind="ExternalInput")
    MASK = nc.dram_tensor("MASK", [128, 128], F32, kind="ExternalInput")
    SELA = nc.dram_tensor("SELA", [128, 8], F32, kind="ExternalInput")
    SELB = nc.dram_tensor("SELB", [128, 8], F32, kind="ExternalInput")
    SELTA = nc.dram_tensor("SELTA", [8, 128], F32, kind="ExternalInput")
    SELTB = nc.dram_tensor("SELTB", [8, 128], F32, kind="ExternalInput")
    ONESF = nc.dram_tensor("ONESF", [128, 128], F32, kind="ExternalInput")
    OUT = nc.dram_tensor("OUT", [T, COUT], F32, kind="ExternalOutput")

    xT3 = xT.ap().rearrange("(ko ki) t -> ki ko t", ki=128)      # [128, KC, T]
    Wq3 = Wq.ap().rearrange("(ko ki) m -> ki ko m", ki=128)      # [128, KC, 512]
    Wk3 = Wk.ap().rearrange("(ko ki) m -> ki ko m", ki=128)
    Wv3 = Wv.ap().rearrange("(ko ki) m -> ki ko m", ki=128)
    Wo3 = Wo.ap().rearrange("(mo mi) n -> mi mo n", mi=128)      # [128, 4, COUT]

    with tile.TileContext(nc) as tc:
        with (
            tc.tile_pool(name="consts", bufs=1) as cpool,
            tc.tile_pool(name="big", bufs=1) as big,
            tc.tile_pool(name="w", bufs=1) as wpool,
            tc.tile_pool(name="xtb", bufs=2) as xpool,
            tc.tile_pool(name="work", bufs=2) as work,
            tc.tile_pool(name="tmp", bufs=2) as tmp,
            tc.tile_pool(name="dram", bufs=1, space="DRAM") as dpool,
            tc.tile_pool(name="psa", bufs=1, space="PSUM") as psa,
            tc.tile_pool(name="psb", bufs=1, space="PSUM") as psb,
            tc.tile_pool(name="psy", bufs=4, space="PSUM") as psy,
        ):
            # ---- constants ----
            mask_sb = cpool.tile([128, 128], F32, tag="mask")
            nc.sync.dma_start(out=mask_sb, in_=MASK[:, :])
            sela_sb = cpool.tile([128, 8], F32R, tag="sela")
            nc.sync.dma_start(out=sela_sb, in_=SELA[:, :].bitcast(F32R))
            selb_sb = cpool.tile([128, 8], F32R, tag="selb")
            nc.sync.dma_start(out=selb_sb, in_=SELB[:, :].bitcast(F32R))
            selta_sb = cpool.tile([8, 128], F32R, tag="selta")
            nc.sync.dma_start(out=selta_sb, in_=SELTA[:, :].bitcast(F32R))
            seltb_sb = cpool.tile([8, 128], F32R, tag="seltb")
            nc.sync.dma_start(out=seltb_sb, in_=SELTB[:, :].bitcast(F32R))
            ones_sb = cpool.tile([128, 64], F32R, tag="ones")
            nc.sync.dma_start(out=ones_sb, in_=ONESF[:, 0:64].bitcast(F32R))
            biasq = cpool.tile([8, 1], F32, tag="biasq")
            nc.vector.memset(biasq, 64.0 * EPS)
            biask = cpool.tile([8, 1], F32, tag="biask")
            nc.vector.memset(biask, EPS)

            khat = big.tile([128, 4, T], F32R, tag="khat")
            vsb = big.tile([128, NKC, 520], F32R, tag="v")
            vsb4 = vsb.rearrange("p n (h e) -> p n h e", e=65)

            def project_qk(w_sb, xtb, dst, ts, is_q, cos_ts=None):
                """Project one 512-t block into dst[:, :, ts] with RoPE+RMS.
                cos_ts: global t slice for the RoPE tables (defaults to ts)."""
                if cos_ts is None:
                    cos_ts = ts
                cos_sb = work.tile([128, 512], F32, tag="cos")
                nc.sync.dma_start(out=cos_sb, in_=COS[:, cos_ts])
                sin_sb = work.tile([128, 512], F32, tag="sin")
                nc.sync.dma_start(out=sin_sb, in_=SIN[:, cos_ts])
                qpa = psa.tile([128, 2, 512], F32, tag="pa", name="qpa")
                qpb = psb.tile([128, 2, 512], F32, tag="pb", name="qpb")
                for cc in range(4):
                    qp_t = qpa if cc < 2 else qpb
                    for k in range(KC):
                        nc.tensor.matmul(
                            qp_t[:, cc % 2, :],
                            w_sb[:, k, 128 * cc:128 * (cc + 1)],
                            xtb[:, k, :],
                            start=(k == 0), stop=(k == KC - 1),
                        )
                # stage to SBUF on ACT (Copy lives in every table set), so
                # rope runs all-SBUF on DVE and PSUM frees early
                qs = tmp.tile([128, 4, 512], F32R, tag="qs", bufs=1)
                nc.scalar.activation(qs[:, 0:2, :], qpa, ACTF.Copy)
                nc.scalar.activation(qs[:, 2:4, :], qpb, ACTF.Copy)
                # unscaled rope into dst (scaled afterwards, once rms known)
                u1 = qs[:, 0::2, :]
                u2 = qs[:, 1::2, :]
                cosb = cos_sb[:, None, :].to_broadcast([128, 2, 512])
                sinb = sin_sb[:, None, :].to_broadcast([128, 2, 512])
                e1 = tmp.tile([128, 2, 512], F32, tag="r512", bufs=2)
                e2 = tmp.tile([128, 2, 512], F32, tag="r512", bufs=2)
                nc.vector.tensor_mul(e1, u1, cosb)
                nc.vector.tensor_mul(e2, u2, sinb)
                nc.vector.tensor_add(dst[:, 0::2, ts], e1, e2)
                e3 = tmp.tile([128, 2, 512], F32, tag="r512", bufs=2)
                e4 = tmp.tile([128, 2, 512], F32, tag="r512", bufs=2)
                nc.vector.tensor_mul(e3, u2, cosb)
                nc.vector.tensor_mul(e4, u1, sinb)
                nc.vector.tensor_sub(dst[:, 1::2, ts], e3, e4)
                # per-head sum of squares (pre-rope == post-rope norms)
                qsq = tmp.tile([128, 4, 512], F32R, tag="qsq", bufs=1)
                nc.vector.tensor_mul(qsq, qs, qs)
                ssq = psy.tile([8, 512], F32, tag="y", name="ssq")
                for cc in range(4):
                    nc.tensor.matmul(
                        ssq,
                        sela_sb if cc < 2 else selb_sb,
                        qsq[:, cc, :],
                        start=(cc == 0), stop=(cc == 3),
                    )
                # rms factor rows [8, 512]: reciprocal(sqrt(.)). All of
                # phase A uses only {Copy, Sqrt} so one table set suffices.
                sq = tmp.tile([8, 512], F32, tag="sq")
                if is_q:  # 1/sqrt(ssq + 64 eps): folds the 1/sqrt(D) scale
                    nc.scalar.activation(sq, ssq, ACTF.Sqrt, bias=biasq,
                                         scale=1.0)
                else:     # 1/sqrt(ssq/64 + eps)
                    nc.scalar.activation(sq, ssq, ACTF.Sqrt, bias=biask,
                                         scale=1.0 / 64.0)
                rr = tmp.tile([8, 512], F32R, tag="rr")
                with nc.allow_low_precision(reason="f32r feed to PE broadcast"):
                    nc.vector.reciprocal(rr, sq)
                # rms scale applied to the roped output, per chunk pair
                for pr in range(2):
                    bq = psy.tile([128, 512], F32, tag="y", name=f"bq{pr}")
                    nc.tensor.matmul(
                        bq, selta_sb if pr == 0 else seltb_sb, rr,
                        start=True, stop=True,
                    )
                    nc.vector.tensor_mul(
                        dst[:, 2 * pr:2 * pr + 2, ts],
                        dst[:, 2 * pr:2 * pr + 2, ts],
                        bq[:, None, :].to_broadcast([128, 2, 512]),
                    )

            # ============ Phase A: Q-hat->DRAM, K-hat, V (per t-block) ===
            nc.sync.dma_start(
                out=vsb4[:, :, :, 64],
                in_=ONESF.ap()[:, 0:8 * NKC].rearrange(
                    "p (n h) -> p n h", h=8).bitcast(F32R))
            qdram = dpool.tile([128, 4, T], F32R, tag="qd")
            wq_sb = wpool.tile([128, KC, 512], F32R, tag="wa")
            nc.sync.dma_start(out=wq_sb, in_=Wq3[:, :, :].bitcast(F32R))
            wv_sb = wpool.tile([128, KC, 512], F32R, tag="wb")
            nc.sync.dma_start(out=wv_sb, in_=Wv3[:, :, :].bitcast(F32R))
            wk_sb = wpool.tile([128, KC, 512], F32R, tag="wc")
            nc.sync.dma_start(out=wk_sb, in_=Wk3[:, :, :].bitcast(F32R))
            for tb in range(NTB):
                ts = slice(512 * tb, 512 * (tb + 1))
                xtb = xpool.tile([128, KC, 512], F32R, tag="xtb")
                nc.sync.dma_start(out=xtb, in_=xT3[:, :, ts].bitcast(F32R))
                qstage = work.tile([128, 4, 512], F32R, tag="qtb", bufs=1)
                project_qk(wq_sb, xtb, qstage, slice(0, 512), is_q=True,
                           cos_ts=ts)
                nc.sync.dma_start(out=qdram[:, :, ts], in_=qstage)
                project_qk(wk_sb, xtb, khat, ts, is_q=False)
                for j in range(4):
                    vp = psy.tile([128, 512], F32, tag="y", name=f"vp{tb}_{j}")
                    for k in range(KC):
                        nc.tensor.matmul(
                            vp,
                            xtb[:, k, 128 * j:128 * (j + 1)],
                            wv_sb[:, k, :],
                            start=(k == 0), stop=(k == KC - 1),
                        )
                    nc.scalar.activation(
                        vsb4[:, 4 * tb + j, :, 0:64],
                        vp.rearrange("p (h d) -> p h d", d=64), ACTF.Copy)

            # ============ Phase B: per tq block: attention + out-proj ====
            wo_sb = wpool.tile([128, 4, COUT], F32R, tag="wb")
            nc.sync.dma_start(out=wo_sb, in_=Wo3[:, :, :].bitcast(F32R))

            for qb in range(NQ):
                tqs = slice(512 * qb, 512 * (qb + 1))
                qtb = work.tile([128, 4, 512], F32R, tag="qtb", bufs=1)
                nc.sync.dma_start(out=qtb, in_=qdram[:, :, tqs])

                yhat = work.tile([128, 4, 512], F32R, tag="yhat", bufs=1)
                for g in range(2):
                    ybank = [psy.tile([65, 512], F32, tag="y",
                                      name=f"y{qb}_{g}_{j_}") for j_ in range(4)]
                    nkc = 4 * (qb + 1)
                    for c in range(nkc):
                        scs = [psa.tile([128, 2, 512], F32, tag="pa", name="scA"),
                               psb.tile([128, 2, 512], F32, tag="pb", name="scB")]
                        for j in range(4):
                            for half in range(2):
                                cc = 2 * g + half
                                nc.tensor.matmul(
                                    scs[j // 2][:, j % 2, :],
                                    khat[32 * j:32 * (j + 1), cc,
                                         128 * c:128 * (c + 1)],
                                    qtb[32 * j:32 * (j + 1), cc, :],
                                    start=(half == 0), stop=(half == 1),
                                    tile_position=(32 * j, 0),
                                )
                        kd = c - 4 * qb
                        first, last = (c == 0), (c == nkc - 1)
                        # for diagonal chunks only columns [128*kd, 512)
                        # are live: exp, mask, and PV all restrict to the
                        # suffix, so the masked prefix is never touched.
                        # (tq columns < 128*kd take no contribution from this
                        # chunk; chunk 0 is always full-width with start=True.)
                        lo = 128 * kd if kd > 0 else 0
                        for pj in range(2):
                            ph = tmp.tile([128, 2, 512], F32R, tag="r512",
                                          bufs=2, name=f"ph{pj}")
                            nc.scalar.activation(
                                ph[:, :, lo:], scs[pj][:, :, lo:], ACTF.Exp)
                            if kd >= 0:  # diagonal boundary strip
                                nc.vector.tensor_mul(
                                    ph[:, :, 128 * kd:128 * (kd + 1)],
                                    ph[:, :, 128 * kd:128 * (kd + 1)],
                                    mask_sb[:, None, :].to_broadcast(
                                        [128, 2, 128]),
                                )
                            for e in range(2):
                                j = 2 * pj + e
                                hloc = 4 * g + j
                                nc.tensor.matmul(
                                    ybank[j][:, lo:],
                                    vsb[:, c, 65 * hloc:65 * hloc + 65],
                                    ph[:, e, lo:],
                                    start=first, stop=last,
                                    skip_group_check=True,
                                )
                    # normalize: yhat rows = y / denom
                    for j in range(4):
                        hloc = 4 * g + j
                        rcp = tmp.tile([128, 512], F32R, tag="s512")
                        with nc.allow_low_precision(reason="f32r for PE bcast"):
                            nc.vector.reciprocal(rcp[64:65, :],
                                                 ybank[j][64:65, :])
                        rb = psa.tile([128, 512], F32, tag="pa",
                                      name=f"rb{qb}_{g}_{j}")
                        nc.tensor.matmul(
                            rb[0:64, :],
                            ones_sb[64:65, :],
                            rcp[64:65, :],
                            start=True, stop=True,
                            tile_position=(64, 0),
                            skip_group_check=True,
                        )
                        rbs = tmp.tile([128, 512], F32, tag="s512")
                        nc.vector.tensor_copy(out=rbs[0:64, :], in_=rb[0:64, :])
                        nc.vector.tensor_mul(
                            yhat[64 * (hloc % 2):64 * (hloc % 2 + 1),
                                 hloc // 2, :],
                            ybank[j][0:64, :],
                            rbs[0:64, :],
                        )
                # out projection for this tq block
                for n in range(NCO):
                    for jt in range(4):
                        op = psy.tile([128, 512], F32, tag="y",
                                      name=f"op{qb}_{n}_{jt}")
                        for m in range(4):
                            nc.tensor.matmul(
                                op,
                                yhat[:, m, 128 * jt:128 * (jt + 1)],
                                wo_sb[:, m, 512 * n:512 * (n + 1)],
                                start=(m == 0), stop=(m == 3),
                            )
                        osb = tmp.tile([128, 512], F32, tag="s512")
                        nc.vector.tensor_copy(out=osb, in_=op)
                        nc.sync.dma_start(
                            out=OUT[512 * qb + 128 * jt:512 * qb + 128 * (jt + 1),
                                    512 * n:512 * (n + 1)],
                            in_=osb)

    nc.finalize()
    return nc


# ======================================================================
# Full-problem harness: 8 cores = 4 batch x 2 head-groups
# ======================================================================
B_FULL, T_FULL, C_FULL, H_FULL = 4, 2048, 1024, 16

_NC_CACHE = {}


def _get_nc():
    if "nc" not in _NC_CACHE:
        _NC_CACHE["nc"] = build_nc(T_FULL, C_FULL, C_FULL)
    return _NC_CACHE["nc"]


def _consts_from_tables(cos, sin):
    """Like make_consts but using the provided RoPE tables.
    cos/sin: [1, 1, T, 32] float32."""
    c = make_consts(T_FULL)
    c["COS"] = np.ascontiguousarray(np.tile(np.asarray(cos)[0, 0].T, (4, 1)))
    c["SIN"] = np.ascontiguousarray(np.tile(np.asarray(sin)[0, 0].T, (4, 1)))
    return c


def make_in_maps(x, cos, sin, Wq, Wk, Wv, Wo):
    x, Wq, Wk, Wv, Wo = (np.asarray(a, dtype=np.float32)
                         for a in (x, Wq, Wk, Wv, Wo))
    consts = _consts_from_tables(cos, sin)
    in_maps = []
    for core in range(8):
        b, hg = core // 2, core % 2
        cols = slice(512 * hg, 512 * (hg + 1))
        in_maps.append(make_core_inputs(
            x[b], Wq[:, cols], Wk[:, cols], Wv[:, cols], Wo[cols, :], consts))
    return in_maps


def gather_out(results):
    out = np.empty((B_FULL, T_FULL, C_FULL), dtype=np.float32)
    for b in range(B_FULL):
        out[b] = results[2 * b]["OUT"] + results[2 * b + 1]["OUT"]
    return out


def kernel(x, cos, sin, Wq, Wk, Wv, Wo):
    from concourse.bass_utils import run_bass_kernel_spmd
    nc = _get_nc()
    in_maps = make_in_maps(x, cos, sin, Wq, Wk, Wv, Wo)
    res = run_bass_kernel_spmd(nc, in_maps, core_ids=list(range(8)))
    return gather_out(res.results)



# revision 8
# speedup vs baseline: 1.2126x; 1.2126x over previous
"""Causal self-attention (RoPE + parameter-free RMSNorm on Q/K) — bf16 kernel.

Sharding: 8 cores = 4 batch x 2 head-groups (8 heads each). Each core computes
its batch element's attention for its 8 heads plus the transposed partial
output projection; the host sums the two head-group partials per batch.

v2 vs v1:
  - bf16 datapath end to end (PSUM accumulation stays fp32): FWL weight
    loads, 2x DVE modes, half the DMA bytes. Inputs packed into one bf16
    const blob + one bf16 xT tensor; output is bf16 and transposed.
  - Q-hat stays in SBUF (v1 staged it through DRAM and re-read per block).
  - Softmax reciprocals batched on DVE ([4,512] per head-group instead of
    8x [1,512]); denominator broadcast via block-diagonal ones matmul.
  - Out-projection is weight-stationary (OUT^T = Wo^T @ yhat), fewer
    LDWEIGHTS; OUT^T DMAs per 128-column chunk.
  - Optional device-side repeat loop (build_nc(repeat=R)) so a single NEFF
    executes R full passes back to back for steady-state timing.

Per-core device layout (D=64, 8 heads):
  Q^T / K^T stored as [128, 4, T] bf16: col j = 128*cc + p,
     cc = 2*(h//4) + half, p = 32*(h%4) + r,  (d = 32*half + r)
  V stored with a ones column per head: [128, T//128, 8*65] bf16; the ones
  column makes the PV matmul also accumulate the softmax denominator (row 64).
  Scores computed transposed: S^T[tk, tq] per head via K=32 row-tiled matmuls;
  softmax runs without max-subtraction (RMS-normed q,k bound |s| <= 8);
  the denominator division folds in before the output projection.
"""

import sys

import numpy as np

for _p in ("/opt/trn_rl_repo",):
    if _p not in sys.path:
        sys.path.insert(0, _p)

import ml_dtypes

import concourse.bass as bass
import concourse.mybir as mybir
import concourse.tile as tile
from concourse import bacc

F32 = mybir.dt.float32
BF16 = mybir.dt.bfloat16
AX = mybir.AluOpType
ACTF = mybir.ActivationFunctionType
BFNP = ml_dtypes.bfloat16

D = 64
NH = 8          # heads per core
CH = NH * D     # 512 head channels per core
EPS = float(np.finfo(np.float32).eps)


def qk_col_perm():
    """perm[j] = plain column (64*h + d) stored at device column j."""
    perm = np.zeros(CH, dtype=np.int64)
    for h in range(NH):
        for half in range(2):
            for r in range(32):
                j = 128 * (2 * (h // 4) + half) + 32 * (h % 4) + r
                perm[j] = 64 * h + 32 * half + r
    return perm


# ----------------------------------------------------------------------
# Const blob layout (bf16). Each entry: name -> (shape, partition dim).
# Packed on host in C order with the partition dim first, so the device
# can slice CONST[0, off:off+size].rearrange("(p n) -> p n", p=P).
# ----------------------------------------------------------------------
def blob_layout(T, CIN, COUT):
    KC = CIN // 128
    return [
        ("WQ", (128, KC * CH)),        # [ki, (ko m)]
        ("WK", (128, KC * CH)),
        ("WV", (128, KC * CH)),
        ("WO", (128, 4 * COUT)),       # [mi, (mo n)]
        ("COS", (128, T)),             # tiled 4x along partitions
        ("SIN", (128, T)),
        ("MASK", (128, 128)),          # [p, j] = p <= j
        ("SELA", (128, 8)),            # ssq reduction, heads 0-3
        ("SELB", (128, 8)),            # heads 4-7
        ("SELTA", (8, 128)),           # rms broadcast, chunks 0,1
        ("SELTB", (8, 128)),           # chunks 2,3
        ("ONES", (128, 128)),          # ones: V ones-columns + denom bcast
    ]


def make_blob_consts(T, cos_t, sin_t):
    """Host-side constant arrays (bf16) keyed by blob entry name.
    cos_t/sin_t: [T, 32] fp32 RoPE tables."""
    cosT = np.ascontiguousarray(cos_t.T)  # [32, T]
    sinT = np.ascontiguousarray(sin_t.T)
    COS = np.tile(cosT, (4, 1))
    SIN = np.tile(sinT, (4, 1))
    p = np.arange(128)[:, None]
    j = np.arange(128)[None, :]
    MASK = (p <= j).astype(np.float32)
    SELA = np.zeros((128, 8), dtype=np.float32)
    SELB = np.zeros((128, 8), dtype=np.float32)
    for pp in range(128):
        SELA[pp, pp // 32] = 1.0
        SELB[pp, 4 + pp // 32] = 1.0
    SELTA = SELA.T.copy()
    SELTB = SELB.T.copy()
    ONES = np.ones((128, 128), dtype=np.float32)
    return dict(COS=COS, SIN=SIN, MASK=MASK, SELA=SELA, SELB=SELB,
                SELTA=SELTA, SELTB=SELTB, ONES=ONES)


def pack_blob(T, CIN, COUT, named):
    """Pack named arrays (host dtype any float) into one bf16 blob [1, N]."""
    chunks = []
    for name, shape in blob_layout(T, CIN, COUT):
        a = np.asarray(named[name], dtype=np.float32)
        assert a.shape == shape, (name, a.shape, shape)
        chunks.append(np.ascontiguousarray(a).astype(BFNP).reshape(-1))
    return np.concatenate(chunks)[None, :]


def blob_offsets(T, CIN, COUT):
    offs = {}
    off = 0
    for name, shape in blob_layout(T, CIN, COUT):
        n = int(np.prod(shape))
        offs[name] = (off, shape)
        off += n
    return offs, off


def build_nc(T, CIN, COUT, repeat=1):
    """Build the Bass program. If repeat > 1, the whole computation runs
    `repeat` times in a device-side loop (for steady-state timing)."""
    assert T % 512 == 0 and CIN % 128 == 0 and COUT % 512 == 0
    KC = CIN // 128        # c_in chunks
    NTB = T // 512         # projection t-blocks == tq blocks
    NQ = T // 512
    NKC = T // 128         # tk chunks
    NO8 = COUT // 128      # out-proj column chunks

    nc = bacc.Bacc()

    offs, blob_n = blob_offsets(T, CIN, COUT)
    XT = nc.dram_tensor("XT", [CIN, T], BF16, kind="ExternalInput")
    CONST = nc.dram_tensor("CONST", [1, blob_n], BF16, kind="ExternalInput")
    OUTT = nc.dram_tensor("OUTT", [COUT, T], BF16, kind="ExternalOutput")

    def cslice(name):
        off, shape = offs[name]
        n = int(np.prod(shape))
        ap = CONST.ap()[0, off:off + n]
        return ap.rearrange("(p n) -> p n", p=shape[0])

    xT3 = XT.ap().rearrange("(ko ki) t -> ki ko t", ki=128)      # [128, KC, T]

    with tile.TileContext(nc) as tc:
        with (
            tc.tile_pool(name="consts", bufs=1) as cpool,
            tc.tile_pool(name="big", bufs=1) as big,
            tc.tile_pool(name="w", bufs=1) as wpool,
            tc.tile_pool(name="xtb", bufs=2) as xpool,
            tc.tile_pool(name="work", bufs=1) as work,
            tc.tile_pool(name="tmp", bufs=2) as tmp,
            tc.tile_pool(name="psa", bufs=1, space="PSUM") as psa,
            tc.tile_pool(name="psb", bufs=1, space="PSUM") as psb,
            tc.tile_pool(name="psy", bufs=4, space="PSUM") as psy,
        ):
            # ---- constants (loaded once, outside the repeat loop) ----
            mask_sb = cpool.tile([128, 128], BF16, tag="mask")
            nc.sync.dma_start(out=mask_sb, in_=cslice("MASK"))
            sela_sb = cpool.tile([128, 8], BF16, tag="sela")
            nc.sync.dma_start(out=sela_sb, in_=cslice("SELA"))
            selb_sb = cpool.tile([128, 8], BF16, tag="selb")
            nc.sync.dma_start(out=selb_sb, in_=cslice("SELB"))
            selta_sb = cpool.tile([8, 128], BF16, tag="selta")
            nc.sync.dma_start(out=selta_sb, in_=cslice("SELTA"))
            seltb_sb = cpool.tile([8, 128], BF16, tag="seltb")
            nc.sync.dma_start(out=seltb_sb, in_=cslice("SELTB"))
            ones_sb = cpool.tile([128, 64], BF16, tag="ones")
            nc.sync.dma_start(out=ones_sb, in_=cslice("ONES")[:, 0:64])
            cos_sb = cpool.tile([128, T], BF16, tag="cos")
            nc.sync.dma_start(out=cos_sb, in_=cslice("COS"))
            sin_sb = cpool.tile([128, T], BF16, tag="sin")
            nc.sync.dma_start(out=sin_sb, in_=cslice("SIN"))
            biasq = cpool.tile([8, 1], F32, tag="biasq")
            nc.vector.memset(biasq, 64.0 * EPS)
            biask = cpool.tile([8, 1], F32, tag="biask")
            nc.vector.memset(biask, EPS)

            khat = big.tile([128, 4, T], BF16, tag="khat")
            qhat = big.tile([128, 4, T], BF16, tag="qhat")
            vsb = big.tile([128, NKC, 520], BF16, tag="v")
            vsb4 = vsb.rearrange("p n (h e) -> p n h e", e=65)

            wq_sb = wpool.tile([128, KC, CH], BF16, tag="wa")
            wk_sb = wpool.tile([128, KC, CH], BF16, tag="wc")
            wv_sb = wpool.tile([128, KC, CH], BF16, tag="wb")
            wo_sb = wpool.tile([128, 4, COUT], BF16, tag="wo")

            def load_weights():
                nc.sync.dma_start(
                    out=wq_sb,
                    in_=cslice("WQ").rearrange("p (k m) -> p k m", k=KC))
                nc.sync.dma_start(
                    out=wk_sb,
                    in_=cslice("WK").rearrange("p (k m) -> p k m", k=KC))
                nc.sync.dma_start(
                    out=wv_sb,
                    in_=cslice("WV").rearrange("p (k m) -> p k m", k=KC))
                nc.sync.dma_start(
                    out=wo_sb,
                    in_=cslice("WO").rearrange("p (m n) -> p m n", m=4))
                nc.sync.dma_start(
                    out=vsb4[:, :, :, 64],
                    in_=cslice("ONES")[:, 0:8 * NKC].rearrange(
                        "p (n h) -> p n h", h=8))

            def project_qk(w_sb, xtb, dst, ts, is_q):
                """Project one 512-t block into dst[:, :, ts] with RoPE+RMS.
                ts is the global t slice (also indexes the RoPE tables)."""
                qpa = psa.tile([128, 2, 512], F32, tag="pa", name="qpa")
                qpb = psb.tile([128, 2, 512], F32, tag="pb", name="qpb")
                for cc in range(4):
                    qp_t = qpa if cc < 2 else qpb
                    for k in range(KC):
                        nc.tensor.matmul(
                            qp_t[:, cc % 2, :],
                            w_sb[:, k, 128 * cc:128 * (cc + 1)],
                            xtb[:, k, :],
                            start=(k == 0), stop=(k == KC - 1),
                        )
                # stage to SBUF bf16 on ACT so rope runs on DVE in 2x mode
                qs = tmp.tile([128, 4, 512], BF16, tag="qs", bufs=1)
                nc.scalar.activation(qs[:, 0:2, :], qpa, ACTF.Copy)
                nc.scalar.activation(qs[:, 2:4, :], qpb, ACTF.Copy)
                # unscaled rope into dst (scaled afterwards, once rms known)
                u1 = qs[:, 0::2, :]
                u2 = qs[:, 1::2, :]
                cosb = cos_sb[:, None, ts].to_broadcast([128, 2, 512])
                sinb = sin_sb[:, None, ts].to_broadcast([128, 2, 512])
                e1 = tmp.tile([128, 2, 512], BF16, tag="r512", bufs=2)
                e2 = tmp.tile([128, 2, 512], BF16, tag="r512", bufs=2)
                nc.vector.tensor_mul(e1, u1, cosb)
                nc.vector.tensor_mul(e2, u2, sinb)
                nc.vector.tensor_add(dst[:, 0::2, ts], e1, e2)
                e3 = tmp.tile([128, 2, 512], BF16, tag="r512", bufs=2)
                e4 = tmp.tile([128, 2, 512], BF16, tag="r512", bufs=2)
                nc.vector.tensor_mul(e3, u2, cosb)
                nc.vector.tensor_mul(e4, u1, sinb)
                nc.vector.tensor_sub(dst[:, 1::2, ts], e3, e4)
                # per-head sum of squares (pre-rope == post-rope norms)
                qsq = tmp.tile([128, 4, 512], BF16, tag="qsq", bufs=1)
                nc.vector.tensor_mul(qsq, qs, qs)
                ssq = psy.tile([8, 512], F32, tag="y", name="ssq")
                for cc in range(4):
                    nc.tensor.matmul(
                        ssq,
                        sela_sb if cc < 2 else selb_sb,
                        qsq[:, cc, :],
                        start=(cc == 0), stop=(cc == 3),
                    )
                # rms factor rows [8, 512]: reciprocal(sqrt(.)). Phase A
                # keeps ACT on the {Copy, Sqrt} table set.
                sq = tmp.tile([8, 512], F32, tag="sq")
                if is_q:  # 1/sqrt(ssq + 64 eps): folds the 1/sqrt(D) scale
                    nc.scalar.activation(sq, ssq, ACTF.Sqrt, bias=biasq,
                                         scale=1.0)
                else:     # 1/sqrt(ssq/64 + eps)
                    nc.scalar.activation(sq, ssq, ACTF.Sqrt, bias=biask,
                                         scale=1.0 / 64.0)
                rr = tmp.tile([8, 512], BF16, tag="rr")
                with nc.allow_low_precision(reason="bf16 rms factors"):
                    nc.vector.reciprocal(rr, sq)
                # rms scale applied to the roped output, per chunk pair
                for pr in range(2):
                    bq = psy.tile([128, 512], F32, tag="y", name=f"bq{pr}")
                    nc.tensor.matmul(
                        bq, selta_sb if pr == 0 else seltb_sb, rr,
                        start=True, stop=True,
                    )
                    bqs = tmp.tile([128, 512], BF16, tag="bqs", bufs=2)
                    nc.scalar.activation(bqs, bq, ACTF.Copy)
                    nc.vector.tensor_mul(
                        dst[:, 2 * pr:2 * pr + 2, ts],
                        dst[:, 2 * pr:2 * pr + 2, ts],
                        bqs[:, None, :].to_broadcast([128, 2, 512]),
                    )

            def body():
                load_weights()
                # ===== Phase A: Q-hat, K-hat, V (per t-block) =====
                for tb in range(NTB):
                    ts = slice(512 * tb, 512 * (tb + 1))
                    xtb = xpool.tile([128, KC, 512], BF16, tag="xtb")
                    nc.sync.dma_start(out=xtb, in_=xT3[:, :, ts])
                    project_qk(wq_sb, xtb, qhat, ts, is_q=True)
                    project_qk(wk_sb, xtb, khat, ts, is_q=False)
                    for j in range(4):
                        vp = psy.tile([128, 512], F32, tag="y",
                                      name=f"vp{tb}_{j}")
                        for k in range(KC):
                            nc.tensor.matmul(
                                vp,
                                xtb[:, k, 128 * j:128 * (j + 1)],
                                wv_sb[:, k, :],
                                start=(k == 0), stop=(k == KC - 1),
                            )
                        nc.scalar.activation(
                            vsb4[:, 4 * tb + j, :, 0:64],
                            vp.rearrange("p (h d) -> p h d", d=64), ACTF.Copy)

                # ===== Phase B: per tq block: attention =====
                yhat = big.tile([128, 4, T], BF16, tag="yhat")
                for qb in range(NQ):
                    tqs = slice(512 * qb, 512 * (qb + 1))
                    for g in range(2):
                        ybank = [psy.tile([65, 512], F32, tag="y",
                                          name=f"y{qb}_{g}_{j_}")
                                 for j_ in range(4)]
                        nkc = 4 * (qb + 1)
                        for c in range(nkc):
                            scs = [psa.tile([128, 2, 512], F32, tag="pa",
                                            name="scA"),
                                   psb.tile([128, 2, 512], F32, tag="pb",
                                            name="scB")]
                            for j in range(4):
                                for half in range(2):
                                    cc = 2 * g + half
                                    nc.tensor.matmul(
                                        scs[j // 2][:, j % 2, :],
                                        khat[32 * j:32 * (j + 1), cc,
                                             128 * c:128 * (c + 1)],
                                        qhat[32 * j:32 * (j + 1), cc, tqs],
                                        start=(half == 0), stop=(half == 1),
                                        tile_position=(32 * j, 0),
                                    )
                            kd = c - 4 * qb
                            first, last = (c == 0), (c == nkc - 1)
                            # diagonal chunks: only columns >= 128*kd live
                            lo = 128 * kd if kd > 0 else 0
                            for pj in range(2):
                                ph = tmp.tile([128, 2, 512], BF16, tag="r512",
                                              bufs=2, name=f"ph{pj}")
                                nc.scalar.activation(
                                    ph[:, :, lo:], scs[pj][:, :, lo:],
                                    ACTF.Exp)
                                if kd >= 0:  # diagonal boundary strip
                                    nc.vector.tensor_mul(
                                        ph[:, :, 128 * kd:128 * (kd + 1)],
                                        ph[:, :, 128 * kd:128 * (kd + 1)],
                                        mask_sb[:, None, :].to_broadcast(
                                            [128, 2, 128]),
                                    )
                                for e in range(2):
                                    j = 2 * pj + e
                                    hloc = 4 * g + j
                                    nc.tensor.matmul(
                                        ybank[j][:, lo:],
                                        vsb[:, c, 65 * hloc:65 * hloc + 65],
                                        ph[:, e, lo:],
                                        start=first, stop=last,
                                        skip_group_check=True,
                                    )
                        # normalize: yhat rows = y / denom (reciprocal of
                        # the ones-column row, broadcast down 64 rows via PE)
                        for j in range(4):
                            hloc = 4 * g + j
                            rcp = tmp.tile([65, 512], BF16, tag="rcp")
                            with nc.allow_low_precision(
                                    reason="bf16 softmax div"):
                                nc.vector.reciprocal(rcp[64:65, :],
                                                     ybank[j][64:65, :])
                            rb = psa.tile([128, 512], F32, tag="pa",
                                          name=f"rb{qb}_{g}_{j}")
                            nc.tensor.matmul(
                                rb[0:64, :],
                                ones_sb[64:65, :],
                                rcp[64:65, :],
                                start=True, stop=True,
                                tile_position=(64, 0),
                                skip_group_check=True,
                            )
                            rbs = tmp.tile([64, 512], BF16, tag="rbs")
                            nc.scalar.activation(rbs, rb[0:64, :], ACTF.Copy)
                            nc.vector.tensor_mul(
                                yhat[64 * (hloc % 2):64 * (hloc % 2 + 1),
                                     hloc // 2, tqs],
                                ybank[j][0:64, :],
                                rbs,
                            )

                # ===== Phase C: transposed out-projection =====
                # OUT^T[n, t] = sum_m Wo[m, n]^T yhat[m, t]; weight-stationary
                for n8 in range(NO8):
                    for qb in range(NQ):
                        tqs = slice(512 * qb, 512 * (qb + 1))
                        op = psy.tile([128, 512], F32, tag="y",
                                      name=f"op{n8}_{qb}")
                        for m in range(4):
                            nc.tensor.matmul(
                                op,
                                wo_sb[:, m, 128 * n8:128 * (n8 + 1)],
                                yhat[:, m, tqs],
                                start=(m == 0), stop=(m == 3),
                            )
                        osb = tmp.tile([128, 512], BF16, tag="osb")
                        nc.vector.tensor_copy(out=osb, in_=op)
                        nc.sync.dma_start(
                            out=OUTT[128 * n8:128 * (n8 + 1), tqs],
                            in_=osb)

            if repeat == 1:
                body()
            else:
                with tc.For_i(0, repeat, 1):
                    body()

    nc.finalize()
    return nc


# ======================================================================
# Full-problem harness: 8 cores = 4 batch x 2 head-groups
# ======================================================================
B_FULL, T_FULL, C_FULL, H_FULL = 4, 2048, 1024, 16

_NC_CACHE = {}


def _get_nc(repeat=1):
    key = ("nc", repeat)
    if key not in _NC_CACHE:
        _NC_CACHE[key] = build_nc(T_FULL, C_FULL, C_FULL, repeat=repeat)
    return _NC_CACHE[key]


def make_in_maps(x, cos, sin, Wq, Wk, Wv, Wo):
    x, Wq, Wk, Wv, Wo = (np.asarray(a, dtype=np.float32)
                         for a in (x, Wq, Wk, Wv, Wo))
    cos_t = np.asarray(cos, dtype=np.float32)[0, 0]   # [T, 32]
    sin_t = np.asarray(sin, dtype=np.float32)[0, 0]
    consts = make_blob_consts(T_FULL, cos_t, sin_t)
    perm = qk_col_perm()
    KC = C_FULL // 128
    in_maps = []
    for core in range(8):
        b, hg = core // 2, core % 2
        cols = slice(512 * hg, 512 * (hg + 1))
        named = dict(consts)
        named["WQ"] = Wq[:, cols][:, perm].reshape(KC, 128, CH) \
            .transpose(1, 0, 2).reshape(128, KC * CH)
        named["WK"] = Wk[:, cols][:, perm].reshape(KC, 128, CH) \
            .transpose(1, 0, 2).reshape(128, KC * CH)
        named["WV"] = Wv[:, cols].reshape(KC, 128, CH) \
            .transpose(1, 0, 2).reshape(128, KC * CH)
        named["WO"] = Wo[cols, :].reshape(4, 128, C_FULL) \
            .transpose(1, 0, 2).reshape(128, 4 * C_FULL)
        blob = pack_blob(T_FULL, C_FULL, C_FULL, named)
        in_maps.append(dict(
            XT=np.ascontiguousarray(x[b].T).astype(BFNP),
            CONST=blob,
        ))
    return in_maps


def gather_out(results):
    out = np.empty((B_FULL, T_FULL, C_FULL), dtype=np.float32)
    for b in range(B_FULL):
        s = (results[2 * b]["OUTT"].astype(np.float32)
             + results[2 * b + 1]["OUTT"].astype(np.float32))
        out[b] = s.T
    return out


def kernel(x, cos, sin, Wq, Wk, Wv, Wo):
    from concourse.bass_utils import run_bass_kernel_spmd
    nc = _get_nc()
    in_maps = make_in_maps(x, cos, sin, Wq, Wk, Wv, Wo)
    res = run_bass_kernel_spmd(nc, in_maps, core_ids=list(range(8)))
    return gather_out(res.results)


# revision 35
# speedup vs baseline: 2.8805x; 2.3754x over previous
"""Causal self-attention (RoPE + parameter-free RMSNorm on Q/K) — bf16 kernel.

Sharding: 8 cores = 4 batch x 2 head-groups (8 heads each). Each core computes
its batch element's attention for its 8 heads plus the transposed partial
output projection; the host sums the two head-group partials per batch.

v2 vs v1:
  - bf16 datapath end to end (PSUM accumulation stays fp32): FWL weight
    loads, 2x DVE modes, half the DMA bytes. Inputs packed into one bf16
    const blob + one bf16 xT tensor; output is bf16 and transposed.
  - Q-hat stays in SBUF (v1 staged it through DRAM and re-read per block).
  - Softmax reciprocals batched on DVE ([4,512] per head-group instead of
    8x [1,512]); denominator broadcast via block-diagonal ones matmul.
  - Out-projection is weight-stationary (OUT^T = Wo^T @ yhat), fewer
    LDWEIGHTS; OUT^T DMAs per 128-column chunk.
  - Optional device-side repeat loop (build_nc(repeat=R)) so a single NEFF
    executes R full passes back to back for steady-state timing.

Per-core device layout (D=64, 8 heads):
  Q^T / K^T stored as [128, 4, T] bf16: col j = 128*cc + p,
     cc = 2*(h//4) + half, p = 32*(h%4) + r,  (d = 32*half + r)
  V stored with a ones column per head: [128, T//128, 8*65] bf16; the ones
  column makes the PV matmul also accumulate the softmax denominator (row 64).
  Scores computed transposed: S^T[tk, tq] per head via K=32 row-tiled matmuls;
  softmax runs without max-subtraction (RMS-normed q,k bound |s| <= 8);
  the denominator division folds in before the output projection.
"""

import sys

import numpy as np

for _p in ("/opt/trn_rl_repo",):
    if _p not in sys.path:
        sys.path.insert(0, _p)

import ml_dtypes

import concourse.bass as bass
import concourse.mybir as mybir
import concourse.tile as tile
from concourse import bacc

F32 = mybir.dt.float32
BF16 = mybir.dt.bfloat16
AX = mybir.AluOpType
ACTF = mybir.ActivationFunctionType
BFNP = ml_dtypes.bfloat16

D = 64
NH = 8          # heads per core
CH = NH * D     # 512 head channels per core
EPS = float(np.finfo(np.float32).eps)


def qk_col_perm():
    """perm[j] = plain column (64*h + d) stored at device column j."""
    perm = np.zeros(CH, dtype=np.int64)
    for h in range(NH):
        for half in range(2):
            for r in range(32):
                j = 128 * (2 * (h // 4) + half) + 32 * (h % 4) + r
                perm[j] = 64 * h + 32 * half + r
    return perm


# ----------------------------------------------------------------------
# Const blob layout (bf16). Each entry: name -> (shape, partition dim).
# Packed on host in C order with the partition dim first, so the device
# can slice CONST[0, off:off+size].rearrange("(p n) -> p n", p=P).
# ----------------------------------------------------------------------
def blob_layout(T, CIN, COUT):
    KC = CIN // 128
    return [
        ("WQ", (128, KC * CH)),        # [ki, (ko m)]
        ("WK", (128, KC * CH)),
        ("WV", (128, KC * CH)),
        ("WO", (128, 4 * COUT)),       # [mi, (mo n)]
        ("COS", (128, T)),             # tiled 4x along partitions
        ("SIN", (128, T)),
        ("MASK", (128, 128)),          # [p, j] = p <= j
        ("SELA", (128, 8)),            # ssq reduction, heads 0-3
        ("SELB", (128, 8)),            # heads 4-7
        ("SELTA", (8, 128)),           # rms broadcast, chunks 0,1
        ("SELTB", (8, 128)),           # chunks 2,3
        ("ONES", (128, 128)),          # ones: V ones-columns + denom bcast
    ]


def make_blob_consts(T, cos_t, sin_t):
    """Host-side constant arrays (bf16) keyed by blob entry name.
    cos_t/sin_t: [T, 32] fp32 RoPE tables."""
    cosT = np.ascontiguousarray(cos_t.T)  # [32, T]
    sinT = np.ascontiguousarray(sin_t.T)
    COS = np.tile(cosT, (4, 1))
    SIN = np.tile(sinT, (4, 1))
    p = np.arange(128)[:, None]
    j = np.arange(128)[None, :]
    MASK = (p <= j).astype(np.float32)
    SELA = np.zeros((128, 8), dtype=np.float32)
    SELB = np.zeros((128, 8), dtype=np.float32)
    for pp in range(128):
        SELA[pp, pp // 32] = 1.0
        SELB[pp, 4 + pp // 32] = 1.0
    SELTA = SELA.T.copy()
    SELTB = SELB.T.copy()
    ONES = np.ones((128, 128), dtype=np.float32)
    return dict(COS=COS, SIN=SIN, MASK=MASK, SELA=SELA, SELB=SELB,
                SELTA=SELTA, SELTB=SELTB, ONES=ONES)


def pack_blob(T, CIN, COUT, named):
    """Pack named arrays (host dtype any float) into one bf16 blob [1, N]."""
    chunks = []
    for name, shape in blob_layout(T, CIN, COUT):
        a = np.asarray(named[name], dtype=np.float32)
        assert a.shape == shape, (name, a.shape, shape)
        chunks.append(np.ascontiguousarray(a).astype(BFNP).reshape(-1))
    return np.concatenate(chunks)[None, :]


def blob_offsets(T, CIN, COUT):
    offs = {}
    off = 0
    for name, shape in blob_layout(T, CIN, COUT):
        n = int(np.prod(shape))
        offs[name] = (off, shape)
        off += n
    return offs, off


def build_nc(T, CIN, COUT, repeat=1, phases="ABC", sub="qkvrsm"):
    """Build the Bass program. If repeat > 1, the whole computation runs
    `repeat` times in a device-side loop (for steady-state timing).
    phases: subset of "ABC" to emit (ablation/debug).
    sub: phase-A pieces: q/k/v projections, r=rope, s=ssq, m=rms-apply."""
    assert T % 512 == 0 and CIN % 128 == 0 and COUT % 512 == 0
    KC = CIN // 128        # c_in chunks
    NTB = T // 512         # projection t-blocks == tq blocks
    NQ = T // 512
    NKC = T // 128         # tk chunks
    NO8 = COUT // 128      # out-proj column chunks

    nc = bacc.Bacc()

    offs, blob_n = blob_offsets(T, CIN, COUT)
    # XT host-packed as [ki, tb, ko, tt] so each t-block load is one
    # contiguous 8KB-per-partition DMA.
    XT = nc.dram_tensor("XT", [128, (CIN // 128) * T], BF16,
                        kind="ExternalInput")
    CONST = nc.dram_tensor("CONST", [1, blob_n], BF16, kind="ExternalInput")
    OUTT = nc.dram_tensor("OUTT", [COUT, T], BF16, kind="ExternalOutput")

    def cslice(name):
        off, shape = offs[name]
        n = int(np.prod(shape))
        ap = CONST.ap()[0, off:off + n]
        return ap.rearrange("(p n) -> p n", p=shape[0])

    def xblock(tb):
        KC_ = CIN // 128
        sl = XT.ap()[:, tb * KC_ * 512:(tb + 1) * KC_ * 512]
        return sl.rearrange("p (k t) -> p k t", k=KC_)

    with tile.TileContext(nc) as tc:
        with (
            tc.tile_pool(name="consts", bufs=1) as cpool,
            tc.tile_pool(name="big", bufs=1) as big,
            tc.tile_pool(name="w", bufs=1) as wpool,
            tc.tile_pool(name="xtb", bufs=2) as xpool,
            tc.tile_pool(name="work", bufs=1) as work,
            tc.tile_pool(name="tmp", bufs=2) as tmp,
            tc.tile_pool(name="psa", bufs=2, space="PSUM") as psa,
            tc.tile_pool(name="psy", bufs=4, space="PSUM") as psy,
        ):
            # ---- constants (loaded once, outside the repeat loop) ----
            mask_sb = cpool.tile([128, 128], BF16, tag="mask")
            nc.sync.dma_start(out=mask_sb, in_=cslice("MASK"))
            sela_sb = cpool.tile([128, 8], BF16, tag="sela")
            nc.sync.dma_start(out=sela_sb, in_=cslice("SELA"))
            selb_sb = cpool.tile([128, 8], BF16, tag="selb")
            nc.sync.dma_start(out=selb_sb, in_=cslice("SELB"))
            selta_sb = cpool.tile([8, 128], BF16, tag="selta")
            nc.sync.dma_start(out=selta_sb, in_=cslice("SELTA"))
            seltb_sb = cpool.tile([8, 128], BF16, tag="seltb")
            nc.sync.dma_start(out=seltb_sb, in_=cslice("SELTB"))
            ones_sb = cpool.tile([128, 64], BF16, tag="ones")
            nc.sync.dma_start(out=ones_sb, in_=cslice("ONES")[:, 0:64])
            cos_sb = cpool.tile([128, T], BF16, tag="cos")
            nc.sync.dma_start(out=cos_sb, in_=cslice("COS"))
            sin_sb = cpool.tile([128, T], BF16, tag="sin")
            nc.sync.dma_start(out=sin_sb, in_=cslice("SIN"))
            biasq = cpool.tile([8, 1], F32, tag="biasq")
            nc.vector.memset(biasq, 64.0 * EPS)
            biask = cpool.tile([8, 1], F32, tag="biask")
            nc.vector.memset(biask, EPS)

            khat = big.tile([128, 4, T], BF16, tag="khat")
            qhat = big.tile([128, 4, T], BF16, tag="qhat")
            vsb = big.tile([128, NKC, 520], BF16, tag="v")
            vsb4 = vsb.rearrange("p n (h e) -> p n h e", e=65)

            wq_sb = wpool.tile([128, KC, CH], BF16, tag="wa")
            wk_sb = wpool.tile([128, KC, CH], BF16, tag="wc")
            wv_sb = wpool.tile([128, KC, CH], BF16, tag="wb")
            wo_sb = wpool.tile([128, 4, COUT], BF16, tag="wo")

            def load_weights():
                nc.sync.dma_start(
                    out=wq_sb,
                    in_=cslice("WQ").rearrange("p (k m) -> p k m", k=KC))
                nc.sync.dma_start(
                    out=wk_sb,
                    in_=cslice("WK").rearrange("p (k m) -> p k m", k=KC))
                nc.sync.dma_start(
                    out=wv_sb,
                    in_=cslice("WV").rearrange("p (k m) -> p k m", k=KC))
                nc.sync.dma_start(
                    out=wo_sb,
                    in_=cslice("WO").rearrange("p (m n) -> p m n", m=4))
                nc.sync.dma_start(
                    out=vsb4[:, :, :, 64],
                    in_=cslice("ONES")[:, 0:8 * NKC].rearrange(
                        "p (n h) -> p n h", h=8))

            def project_qk(w_sb, xtb, dst, ts, is_q):
                """Project one 512-t block into dst[:, :, ts] with RoPE+RMS.
                ts is the global t slice (also indexes the RoPE tables)."""
                qpa = psa.tile([128, 2, 512], F32, tag="pa", name="qpa")
                qpb = psa.tile([128, 2, 512], F32, tag="pa", name="qpb")
                for cc in range(4):
                    qp_t = qpa if cc < 2 else qpb
                    for k in range(KC):
                        nc.tensor.matmul(
                            qp_t[:, cc % 2, :],
                            w_sb[:, k, 128 * cc:128 * (cc + 1)],
                            xtb[:, k, :],
                            start=(k == 0), stop=(k == KC - 1),
                        )
                # stage to SBUF bf16 on ACT so rope runs on DVE in 2x mode
                qs = tmp.tile([128, 4, 512], BF16, tag="qs", bufs=1)
                nc.scalar.activation(qs[:, 0:2, :], qpa, ACTF.Copy)
                nc.scalar.activation(qs[:, 2:4, :], qpb, ACTF.Copy)
                # unscaled rope into dst (scaled afterwards, once rms known)
                if "r" in sub:
                    u1 = qs[:, 0::2, :]
                    u2 = qs[:, 1::2, :]
                    cosb = cos_sb[:, None, ts].to_broadcast([128, 2, 512])
                    sinb = sin_sb[:, None, ts].to_broadcast([128, 2, 512])
                    e1 = tmp.tile([128, 2, 512], BF16, tag="r512", bufs=2)
                    e2 = tmp.tile([128, 2, 512], BF16, tag="r512", bufs=2)
                    nc.vector.tensor_mul(e1, u1, cosb)
                    nc.vector.tensor_mul(e2, u2, sinb)
                    nc.vector.tensor_add(dst[:, 0::2, ts], e1, e2)
                    e3 = tmp.tile([128, 2, 512], BF16, tag="r512", bufs=2)
                    e4 = tmp.tile([128, 2, 512], BF16, tag="r512", bufs=2)
                    nc.vector.tensor_mul(e3, u2, cosb)
                    nc.vector.tensor_mul(e4, u1, sinb)
                    nc.vector.tensor_sub(dst[:, 1::2, ts], e3, e4)
                else:
                    nc.vector.tensor_copy(out=dst[:, :, ts], in_=qs)
                if "s" not in sub:
                    return
                # per-head sum of squares (pre-rope == post-rope norms)
                qsq = tmp.tile([128, 4, 512], BF16, tag="qsq", bufs=1)
                nc.vector.tensor_mul(qsq, qs, qs)
                ssq = psy.tile([8, 512], F32, tag="y", name="ssq")
                for cc in range(4):
                    nc.tensor.matmul(
                        ssq,
                        sela_sb if cc < 2 else selb_sb,
                        qsq[:, cc, :],
                        start=(cc == 0), stop=(cc == 3),
                    )
                # rms factor rows [8, 512]: reciprocal(sqrt(.)). Phase A
                # keeps ACT on the {Copy, Sqrt} table set.
                sq = tmp.tile([8, 512], F32, tag="sq")
                if is_q:  # 1/sqrt(ssq + 64 eps): folds the 1/sqrt(D) scale
                    nc.scalar.activation(sq, ssq, ACTF.Sqrt, bias=biasq,
                                         scale=1.0)
                else:     # 1/sqrt(ssq/64 + eps)
                    nc.scalar.activation(sq, ssq, ACTF.Sqrt, bias=biask,
                                         scale=1.0 / 64.0)
                rr = tmp.tile([8, 512], BF16, tag="rr")
                with nc.allow_low_precision(reason="bf16 rms factors"):
                    nc.vector.reciprocal(rr, sq)
                if "m" not in sub:
                    return
                # rms scale applied to the roped output, per chunk pair
                for pr in range(2):
                    bq = psy.tile([128, 512], F32, tag="y", name=f"bq{pr}")
                    nc.tensor.matmul(
                        bq, selta_sb if pr == 0 else seltb_sb, rr,
                        start=True, stop=True,
                    )
                    bqs = tmp.tile([128, 512], BF16, tag="bqs", bufs=2)
                    nc.scalar.activation(bqs, bq, ACTF.Copy)
                    nc.vector.tensor_mul(
                        dst[:, 2 * pr:2 * pr + 2, ts],
                        dst[:, 2 * pr:2 * pr + 2, ts],
                        bqs[:, None, :].to_broadcast([128, 2, 512]),
                    )

            def body():
                load_weights()
                # ===== Phase A: Q-hat, K-hat, V (per t-block) =====
                for tb in range(NTB if "A" in phases else 0):
                    ts = slice(512 * tb, 512 * (tb + 1))
                    xtb = xpool.tile([128, KC, 512], BF16, tag="xtb")
                    nc.sync.dma_start(out=xtb, in_=xblock(tb))
                    if "q" in sub:
                        project_qk(wq_sb, xtb, qhat, ts, is_q=True)
                    if "k" in sub:
                        project_qk(wk_sb, xtb, khat, ts, is_q=False)
                    for j in range(4 if "v" in sub else 0):
                        vp = psy.tile([128, 512], F32, tag="y",
                                      name=f"vp{tb}_{j}")
                        for k in range(KC):
                            nc.tensor.matmul(
                                vp,
                                xtb[:, k, 128 * j:128 * (j + 1)],
                                wv_sb[:, k, :],
                                start=(k == 0), stop=(k == KC - 1),
                            )
                        nc.scalar.activation(
                            vsb4[:, 4 * tb + j, :, 0:64],
                            vp.rearrange("p (h d) -> p h d", d=64), ACTF.Copy)

                # ===== Phase B: per tq block: attention =====
                yhat = big.tile([128, 4, T], BF16, tag="yhat")
                for qb in range(NQ if "B" in phases else 0):
                    tqs = slice(512 * qb, 512 * (qb + 1))
                    for g in range(2):
                        ybank = [psy.tile([65, 512], F32, tag="y",
                                          name=f"y{qb}_{g}_{j_}")
                                 for j_ in range(4)]
                        nkc = 4 * (qb + 1)

                        def emit_pv(c, phs, lo, first, last):
                            for pj in range(2):
                                for e in range(2):
                                    j = 2 * pj + e
                                    hloc = 4 * g + j
                                    nc.tensor.matmul(
                                        ybank[j][:, lo:],
                                        vsb[:, c, 65 * hloc:65 * hloc + 65],
                                        phs[pj][:, e, lo:],
                                        start=first, stop=last,
                                        skip_group_check=True,
                                    )

                        # software-pipelined: PV runs two chunks behind the
                        # scores+exp so the PE never waits on the ACT exp.
                        pending = []
                        for c in range(nkc):
                            scs = [psa.tile([128, 2, 512], F32, tag="pa",
                                            name="scA"),
                                   psa.tile([128, 2, 512], F32, tag="pa",
                                            name="scB")]
                            if "F" in sub:
                                # timing-ablation only: WRONG results —
                                # one full-K MM per scs half instead of 8
                                # row-tiled head MMs.
                                for pj in range(2):
                                    for half in range(2):
                                        cc = 2 * g + half
                                        nc.tensor.matmul(
                                            scs[pj][:, pj, :],
                                            khat[:, cc,
                                                 128 * c:128 * (c + 1)],
                                            qhat[:, cc, tqs],
                                            start=(half == 0),
                                            stop=(half == 1),
                                        )
                            else:
                                for j in range(4):
                                    for half in range(2):
                                        cc = 2 * g + half
                                        nc.tensor.matmul(
                                            scs[j // 2][:, j % 2, :],
                                            khat[32 * j:32 * (j + 1), cc,
                                                 128 * c:128 * (c + 1)],
                                            qhat[32 * j:32 * (j + 1), cc,
                                                 tqs],
                                            start=(half == 0),
                                            stop=(half == 1),
                                            tile_position=(32 * j, 0),
                                        )
                            kd = c - 4 * qb
                            # diagonal chunks: only columns >= 128*kd live
                            lo = 128 * kd if kd > 0 else 0
                            phs = []
                            for pj in range(2):
                                ph = tmp.tile([128, 2, 512], BF16, tag="ph",
                                              bufs=6, name=f"ph{pj}")
                                nc.scalar.activation(
                                    ph[:, :, lo:], scs[pj][:, :, lo:],
                                    ACTF.Exp)
                                if kd >= 0 and "M" not in sub:
                                    # diagonal boundary strip
                                    nc.vector.tensor_mul(
                                        ph[:, :, 128 * kd:128 * (kd + 1)],
                                        ph[:, :, 128 * kd:128 * (kd + 1)],
                                        mask_sb[:, None, :].to_broadcast(
                                            [128, 2, 128]),
                                    )
                                phs.append(ph)
                            pending.append((c, phs, lo, c == 0,
                                            c == nkc - 1))
                            if len(pending) > 2:
                                emit_pv(*pending.pop(0))
                        for p in pending:
                            emit_pv(*p)
                        # normalize: yhat rows = y / denom (reciprocal of
                        # the ones-column row, broadcast down 64 rows via
                        # PE). Batched per engine so no stream stalls:
                        # 4 recips (DVE), 2 bcast matmuls of 2 heads each
                        # (PE), 2 copies (ACT), 4 muls (DVE).
                        rcps = []
                        for j in range(4):
                            rcp = tmp.tile([65, 512], BF16, tag="rcp",
                                           bufs=4)
                            with nc.allow_low_precision(
                                    reason="bf16 softmax div"):
                                nc.vector.reciprocal(rcp[64:65, :],
                                                     ybank[j][64:65, :])
                            rcps.append(rcp)
                        rbss = []
                        for pj in range(2):
                            rb = psa.tile([128, 2, 512], F32, tag="pa",
                                          name=f"rb{qb}_{g}_{pj}")
                            for e in range(2):
                                nc.tensor.matmul(
                                    rb[0:64, e, :],
                                    ones_sb[64:65, :],
                                    rcps[2 * pj + e][64:65, :],
                                    start=True, stop=True,
                                    tile_position=(64, 0),
                                    skip_group_check=True,
                                )
                            rbs = tmp.tile([64, 2, 512], BF16, tag="rbs",
                                           bufs=2)
                            nc.scalar.activation(rbs, rb[0:64, :, :],
                                                 ACTF.Copy)
                            rbss.append(rbs)
                        for j in range(4):
                            hloc = 4 * g + j
                            nc.vector.tensor_mul(
                                yhat[64 * (hloc % 2):64 * (hloc % 2 + 1),
                                     hloc // 2, tqs],
                                ybank[j][0:64, :],
                                rbss[j // 2][:, j % 2, :],
                            )

                    # ==== transposed out-projection for this tq block ====
                    # OUT^T[n, t] = sum_m Wo[m, n]^T yhat[m, t]
                    for n8 in range(NO8 if "C" in phases else 0):
                        op = psy.tile([128, 512], F32, tag="y",
                                      name=f"op{n8}_{qb}")
                        for m in range(4):
                            nc.tensor.matmul(
                                op,
                                wo_sb[:, m, 128 * n8:128 * (n8 + 1)],
                                yhat[:, m, tqs],
                                start=(m == 0), stop=(m == 3),
                            )
                        osb = tmp.tile([128, 512], BF16, tag="osb")
                        nc.vector.tensor_copy(out=osb, in_=op)
                        nc.sync.dma_start(
                            out=OUTT[128 * n8:128 * (n8 + 1), tqs],
                            in_=osb)

            # python-unrolled repeat: the Tile For_i back-edge costs ~600us
            # per iteration on this runtime, so unroll instead.
            for _ in range(repeat):
                body()

    nc.finalize()
    return nc


# ======================================================================
# Full-problem harness: 8 cores = 4 batch x 2 head-groups
# ======================================================================
B_FULL, T_FULL, C_FULL, H_FULL = 4, 2048, 1024, 16

_NC_CACHE = {}


def _get_nc(repeat=1, phases="ABC", sub="qkvrsm"):
    key = ("nc", repeat, phases, sub)
    if key not in _NC_CACHE:
        _NC_CACHE[key] = build_nc(T_FULL, C_FULL, C_FULL, repeat=repeat,
                                  phases=phases, sub=sub)
    return _NC_CACHE[key]


def make_in_maps(x, cos, sin, Wq, Wk, Wv, Wo):
    x, Wq, Wk, Wv, Wo = (np.asarray(a, dtype=np.float32)
                         for a in (x, Wq, Wk, Wv, Wo))
    cos_t = np.asarray(cos, dtype=np.float32)[0, 0]   # [T, 32]
    sin_t = np.asarray(sin, dtype=np.float32)[0, 0]
    consts = make_blob_consts(T_FULL, cos_t, sin_t)
    perm = qk_col_perm()
    KC = C_FULL // 128
    in_maps = []
    for core in range(8):
        b, hg = core // 2, core % 2
        cols = slice(512 * hg, 512 * (hg + 1))
        named = dict(consts)
        named["WQ"] = Wq[:, cols][:, perm].reshape(KC, 128, CH) \
            .transpose(1, 0, 2).reshape(128, KC * CH)
        named["WK"] = Wk[:, cols][:, perm].reshape(KC, 128, CH) \
            .transpose(1, 0, 2).reshape(128, KC * CH)
        named["WV"] = Wv[:, cols].reshape(KC, 128, CH) \
            .transpose(1, 0, 2).reshape(128, KC * CH)
        named["WO"] = Wo[cols, :].reshape(4, 128, C_FULL) \
            .transpose(1, 0, 2).reshape(128, 4 * C_FULL)
        blob = pack_blob(T_FULL, C_FULL, C_FULL, named)
        # [ki, tb, ko, tt] packing: row ki holds, per t-block, all ko
        # chunks' 512-t slices contiguously.
        xt = x[b].T.reshape(KC, 128, T_FULL // 512, 512)
        xt = xt.transpose(1, 2, 0, 3).reshape(128, KC * T_FULL)
        in_maps.append(dict(
            XT=np.ascontiguousarray(xt).astype(BFNP),
            CONST=blob,
        ))
    return in_maps


def gather_out(results):
    out = np.empty((B_FULL, T_FULL, C_FULL), dtype=np.float32)
    for b in range(B_FULL):
        s = (results[2 * b]["OUTT"].astype(np.float32)
             + results[2 * b + 1]["OUTT"].astype(np.float32))
        out[b] = s.T
    return out


def kernel(x, cos, sin, Wq, Wk, Wv, Wo):
    from concourse.bass_utils import run_bass_kernel_spmd
    nc = _get_nc()
    in_maps = make_in_maps(x, cos, sin, Wq, Wk, Wv, Wo)
    res = run_bass_kernel_spmd(nc, in_maps, core_ids=list(range(8)))
    return gather_out(res.results)


# revision 37
# speedup vs baseline: 2.8985x; 1.0063x over previous
"""Causal self-attention (RoPE + parameter-free RMSNorm on Q/K) — bf16 kernel.

Sharding: 8 cores = 4 batch x 2 head-groups (8 heads each). Each core computes
its batch element's attention for its 8 heads plus the transposed partial
output projection; the host sums the two head-group partials per batch.

v2 vs v1:
  - bf16 datapath end to end (PSUM accumulation stays fp32): FWL weight
    loads, 2x DVE modes, half the DMA bytes. Inputs packed into one bf16
    const blob + one bf16 xT tensor; output is bf16 and transposed.
  - Q-hat stays in SBUF (v1 staged it through DRAM and re-read per block).
  - Softmax reciprocals batched on DVE ([4,512] per head-group instead of
    8x [1,512]); denominator broadcast via block-diagonal ones matmul.
  - Out-projection is weight-stationary (OUT^T = Wo^T @ yhat), fewer
    LDWEIGHTS; OUT^T DMAs per 128-column chunk.
  - Optional device-side repeat loop (build_nc(repeat=R)) so a single NEFF
    executes R full passes back to back for steady-state timing.

Per-core device layout (D=64, 8 heads):
  Q^T / K^T stored as [128, 4, T] bf16: col j = 128*cc + p,
     cc = 2*(h//4) + half, p = 32*(h%4) + r,  (d = 32*half + r)
  V stored with a ones column per head: [128, T//128, 8*65] bf16; the ones
  column makes the PV matmul also accumulate the softmax denominator (row 64).
  Scores computed transposed: S^T[tk, tq] per head via K=32 row-tiled matmuls;
  softmax runs without max-subtraction (RMS-normed q,k bound |s| <= 8);
  the denominator division folds in before the output projection.
"""

import sys

import numpy as np

for _p in ("/opt/trn_rl_repo",):
    if _p not in sys.path:
        sys.path.insert(0, _p)

import ml_dtypes

import concourse.bass as bass
import concourse.mybir as mybir
import concourse.tile as tile
from concourse import bacc

F32 = mybir.dt.float32
BF16 = mybir.dt.bfloat16
AX = mybir.AluOpType
ACTF = mybir.ActivationFunctionType
BFNP = ml_dtypes.bfloat16

D = 64
NH = 8          # heads per core
CH = NH * D     # 512 head channels per core
EPS = float(np.finfo(np.float32).eps)


def qk_col_perm():
    """perm[j] = plain column (64*h + d) stored at device column j."""
    perm = np.zeros(CH, dtype=np.int64)
    for h in range(NH):
        for half in range(2):
            for r in range(32):
                j = 128 * (2 * (h // 4) + half) + 32 * (h % 4) + r
                perm[j] = 64 * h + 32 * half + r
    return perm


# ----------------------------------------------------------------------
# Const blob layout (bf16). Each entry: name -> (shape, partition dim).
# Packed on host in C order with the partition dim first, so the device
# can slice CONST[0, off:off+size].rearrange("(p n) -> p n", p=P).
# ----------------------------------------------------------------------
def blob_layout(T, CIN, COUT):
    KC = CIN // 128
    return [
        ("WQ", (128, KC * CH)),        # [ki, (ko m)]
        ("WK", (128, KC * CH)),
        ("WV", (128, KC * CH)),
        ("WO", (128, 4 * COUT)),       # [mi, (mo n)]
        ("COS", (128, T)),             # tiled 4x along partitions
        ("SIN", (128, T)),
        ("MASK", (128, 128)),          # [p, j] = p <= j
        ("SELA", (128, 8)),            # ssq reduction, heads 0-3
        ("SELB", (128, 8)),            # heads 4-7
        ("SELTA", (8, 128)),           # rms broadcast, chunks 0,1
        ("SELTB", (8, 128)),           # chunks 2,3
        ("ONES", (128, 128)),          # ones: V ones-columns + denom bcast
    ]


def make_blob_consts(T, cos_t, sin_t):
    """Host-side constant arrays (bf16) keyed by blob entry name.
    cos_t/sin_t: [T, 32] fp32 RoPE tables."""
    cosT = np.ascontiguousarray(cos_t.T)  # [32, T]
    sinT = np.ascontiguousarray(sin_t.T)
    COS = np.tile(cosT, (4, 1))
    SIN = np.tile(sinT, (4, 1))
    p = np.arange(128)[:, None]
    j = np.arange(128)[None, :]
    MASK = (p <= j).astype(np.float32)
    SELA = np.zeros((128, 8), dtype=np.float32)
    SELB = np.zeros((128, 8), dtype=np.float32)
    for pp in range(128):
        SELA[pp, pp // 32] = 1.0
        SELB[pp, 4 + pp // 32] = 1.0
    SELTA = SELA.T.copy()
    SELTB = SELB.T.copy()
    ONES = np.ones((128, 128), dtype=np.float32)
    return dict(COS=COS, SIN=SIN, MASK=MASK, SELA=SELA, SELB=SELB,
                SELTA=SELTA, SELTB=SELTB, ONES=ONES)


def pack_blob(T, CIN, COUT, named):
    """Pack named arrays (host dtype any float) into one bf16 blob [1, N]."""
    chunks = []
    for name, shape in blob_layout(T, CIN, COUT):
        a = np.asarray(named[name], dtype=np.float32)
        assert a.shape == shape, (name, a.shape, shape)
        chunks.append(np.ascontiguousarray(a).astype(BFNP).reshape(-1))
    return np.concatenate(chunks)[None, :]


def blob_offsets(T, CIN, COUT):
    offs = {}
    off = 0
    for name, shape in blob_layout(T, CIN, COUT):
        n = int(np.prod(shape))
        offs[name] = (off, shape)
        off += n
    return offs, off


def build_nc(T, CIN, COUT, repeat=1, phases="ABC", sub="qkvrsm"):
    """Build the Bass program. If repeat > 1, the whole computation runs
    `repeat` times in a device-side loop (for steady-state timing).
    phases: subset of "ABC" to emit (ablation/debug).
    sub: phase-A pieces: q/k/v projections, r=rope, s=ssq, m=rms-apply."""
    assert T % 512 == 0 and CIN % 128 == 0 and COUT % 512 == 0
    KC = CIN // 128        # c_in chunks
    NTB = T // 512         # projection t-blocks == tq blocks
    NQ = T // 512
    NKC = T // 128         # tk chunks
    NO8 = COUT // 128      # out-proj column chunks

    nc = bacc.Bacc()

    offs, blob_n = blob_offsets(T, CIN, COUT)
    # XT host-packed as [ki, tb, ko, tt] so each t-block load is one
    # contiguous 8KB-per-partition DMA.
    XT = nc.dram_tensor("XT", [128, (CIN // 128) * T], BF16,
                        kind="ExternalInput")
    CONST = nc.dram_tensor("CONST", [1, blob_n], BF16, kind="ExternalInput")
    OUTT = nc.dram_tensor("OUTT", [COUT, T], BF16, kind="ExternalOutput")

    def cslice(name):
        off, shape = offs[name]
        n = int(np.prod(shape))
        ap = CONST.ap()[0, off:off + n]
        return ap.rearrange("(p n) -> p n", p=shape[0])

    def xblock(tb):
        KC_ = CIN // 128
        sl = XT.ap()[:, tb * KC_ * 512:(tb + 1) * KC_ * 512]
        return sl.rearrange("p (k t) -> p k t", k=KC_)

    with tile.TileContext(nc) as tc:
        with (
            tc.tile_pool(name="consts", bufs=1) as cpool,
            tc.tile_pool(name="big", bufs=1) as big,
            tc.tile_pool(name="w", bufs=1) as wpool,
            tc.tile_pool(name="xtb", bufs=2) as xpool,
            tc.tile_pool(name="work", bufs=1) as work,
            tc.tile_pool(name="tmp", bufs=2) as tmp,
            tc.tile_pool(name="psa", bufs=2, space="PSUM") as psa,
            tc.tile_pool(name="psy", bufs=4, space="PSUM") as psy,
        ):
            # ---- constants (loaded once, outside the repeat loop) ----
            mask_sb = cpool.tile([128, 128], BF16, tag="mask")
            nc.sync.dma_start(out=mask_sb, in_=cslice("MASK"))
            sela_sb = cpool.tile([128, 8], BF16, tag="sela")
            nc.sync.dma_start(out=sela_sb, in_=cslice("SELA"))
            selb_sb = cpool.tile([128, 8], BF16, tag="selb")
            nc.sync.dma_start(out=selb_sb, in_=cslice("SELB"))
            selta_sb = cpool.tile([8, 128], BF16, tag="selta")
            nc.sync.dma_start(out=selta_sb, in_=cslice("SELTA"))
            seltb_sb = cpool.tile([8, 128], BF16, tag="seltb")
            nc.sync.dma_start(out=seltb_sb, in_=cslice("SELTB"))
            ones_sb = cpool.tile([128, 64], BF16, tag="ones")
            nc.sync.dma_start(out=ones_sb, in_=cslice("ONES")[:, 0:64])
            cos_sb = cpool.tile([128, T], BF16, tag="cos")
            nc.sync.dma_start(out=cos_sb, in_=cslice("COS"))
            sin_sb = cpool.tile([128, T], BF16, tag="sin")
            nc.sync.dma_start(out=sin_sb, in_=cslice("SIN"))
            biasq = cpool.tile([8, 1], F32, tag="biasq")
            nc.vector.memset(biasq, 64.0 * EPS)
            biask = cpool.tile([8, 1], F32, tag="biask")
            nc.vector.memset(biask, EPS)

            khat = big.tile([128, 4, T], BF16, tag="khat")
            qhat = big.tile([128, 4, T], BF16, tag="qhat")
            vsb = big.tile([128, NKC, 520], BF16, tag="v")
            vsb4 = vsb.rearrange("p n (h e) -> p n h e", e=65)

            wq_sb = wpool.tile([128, KC, CH], BF16, tag="wa")
            wk_sb = wpool.tile([128, KC, CH], BF16, tag="wc")
            wv_sb = wpool.tile([128, KC, CH], BF16, tag="wb")
            wo_sb = wpool.tile([128, 4, COUT], BF16, tag="wo")

            def load_weights():
                nc.sync.dma_start(
                    out=wq_sb,
                    in_=cslice("WQ").rearrange("p (k m) -> p k m", k=KC))
                nc.sync.dma_start(
                    out=wk_sb,
                    in_=cslice("WK").rearrange("p (k m) -> p k m", k=KC))
                nc.sync.dma_start(
                    out=wv_sb,
                    in_=cslice("WV").rearrange("p (k m) -> p k m", k=KC))
                nc.sync.dma_start(
                    out=wo_sb,
                    in_=cslice("WO").rearrange("p (m n) -> p m n", m=4))
                nc.sync.dma_start(
                    out=vsb4[:, :, :, 64],
                    in_=cslice("ONES")[:, 0:8 * NKC].rearrange(
                        "p (n h) -> p n h", h=8))

            def project_qk(w_sb, xtb, dst, ts, is_q):
                """Project one 512-t block into dst[:, :, ts] with RoPE+RMS.
                ts is the global t slice (also indexes the RoPE tables)."""
                qpa = psa.tile([128, 2, 512], F32, tag="pa", name="qpa")
                qpb = psa.tile([128, 2, 512], F32, tag="pa", name="qpb")
                for cc in range(4):
                    qp_t = qpa if cc < 2 else qpb
                    for k in range(KC):
                        nc.tensor.matmul(
                            qp_t[:, cc % 2, :],
                            w_sb[:, k, 128 * cc:128 * (cc + 1)],
                            xtb[:, k, :],
                            start=(k == 0), stop=(k == KC - 1),
                        )
                # stage to SBUF bf16 on ACT so rope runs on DVE in 2x mode
                qs = tmp.tile([128, 4, 512], BF16, tag="qs", bufs=1)
                nc.scalar.activation(qs[:, 0:2, :], qpa, ACTF.Copy)
                nc.scalar.activation(qs[:, 2:4, :], qpb, ACTF.Copy)
                # unscaled rope into dst (scaled afterwards, once rms known)
                if "r" in sub:
                    u1 = qs[:, 0::2, :]
                    u2 = qs[:, 1::2, :]
                    cosb = cos_sb[:, None, ts].to_broadcast([128, 2, 512])
                    sinb = sin_sb[:, None, ts].to_broadcast([128, 2, 512])
                    e1 = tmp.tile([128, 2, 512], BF16, tag="r512", bufs=2)
                    e2 = tmp.tile([128, 2, 512], BF16, tag="r512", bufs=2)
                    nc.vector.tensor_mul(e1, u1, cosb)
                    nc.vector.tensor_mul(e2, u2, sinb)
                    nc.vector.tensor_add(dst[:, 0::2, ts], e1, e2)
                    e3 = tmp.tile([128, 2, 512], BF16, tag="r512", bufs=2)
                    e4 = tmp.tile([128, 2, 512], BF16, tag="r512", bufs=2)
                    nc.vector.tensor_mul(e3, u2, cosb)
                    nc.vector.tensor_mul(e4, u1, sinb)
                    nc.vector.tensor_sub(dst[:, 1::2, ts], e3, e4)
                else:
                    nc.vector.tensor_copy(out=dst[:, :, ts], in_=qs)
                if "s" not in sub:
                    return
                # per-head sum of squares (pre-rope == post-rope norms)
                qsq = tmp.tile([128, 4, 512], BF16, tag="qsq", bufs=1)
                nc.vector.tensor_mul(qsq, qs, qs)
                ssq = psy.tile([8, 512], F32, tag="y", name="ssq")
                for cc in range(4):
                    nc.tensor.matmul(
                        ssq,
                        sela_sb if cc < 2 else selb_sb,
                        qsq[:, cc, :],
                        start=(cc == 0), stop=(cc == 3),
                    )
                # rms factor rows [8, 512]: reciprocal(sqrt(.)). Phase A
                # keeps ACT on the {Copy, Sqrt} table set.
                sq = tmp.tile([8, 512], F32, tag="sq")
                if is_q:  # 1/sqrt(ssq + 64 eps): folds the 1/sqrt(D) scale
                    nc.scalar.activation(sq, ssq, ACTF.Sqrt, bias=biasq,
                                         scale=1.0)
                else:     # 1/sqrt(ssq/64 + eps)
                    nc.scalar.activation(sq, ssq, ACTF.Sqrt, bias=biask,
                                         scale=1.0 / 64.0)
                rr = tmp.tile([8, 512], BF16, tag="rr")
                with nc.allow_low_precision(reason="bf16 rms factors"):
                    nc.vector.reciprocal(rr, sq)
                if "m" not in sub:
                    return
                # rms scale applied to the roped output, per chunk pair
                for pr in range(2):
                    bq = psy.tile([128, 512], F32, tag="y", name=f"bq{pr}")
                    nc.tensor.matmul(
                        bq, selta_sb if pr == 0 else seltb_sb, rr,
                        start=True, stop=True,
                    )
                    bqs = tmp.tile([128, 512], BF16, tag="bqs", bufs=2)
                    nc.scalar.activation(bqs, bq, ACTF.Copy)
                    nc.vector.tensor_mul(
                        dst[:, 2 * pr:2 * pr + 2, ts],
                        dst[:, 2 * pr:2 * pr + 2, ts],
                        bqs[:, None, :].to_broadcast([128, 2, 512]),
                    )

            def body():
                if "W" not in sub:
                    load_weights()
                # ===== Phase A: Q-hat, K-hat, V (per t-block) =====
                for tb in range(NTB if "A" in phases else 0):
                    ts = slice(512 * tb, 512 * (tb + 1))
                    xtb = xpool.tile([128, KC, 512], BF16, tag="xtb")
                    nc.sync.dma_start(out=xtb, in_=xblock(tb))
                    if "q" in sub:
                        project_qk(wq_sb, xtb, qhat, ts, is_q=True)
                    if "k" in sub:
                        project_qk(wk_sb, xtb, khat, ts, is_q=False)
                    for j in range(4 if "v" in sub else 0):
                        vp = psy.tile([128, 512], F32, tag="y",
                                      name=f"vp{tb}_{j}")
                        for k in range(KC):
                            nc.tensor.matmul(
                                vp,
                                xtb[:, k, 128 * j:128 * (j + 1)],
                                wv_sb[:, k, :],
                                start=(k == 0), stop=(k == KC - 1),
                            )
                        nc.scalar.activation(
                            vsb4[:, 4 * tb + j, :, 0:64],
                            vp.rearrange("p (h d) -> p h d", d=64), ACTF.Copy)

                # ===== Phase B: per tq block: attention =====
                yhat = big.tile([128, 4, T], BF16, tag="yhat")
                for qb in range(NQ if "B" in phases else 0):
                    tqs = slice(512 * qb, 512 * (qb + 1))
                    for g in range(2):
                        ybank = [psy.tile([65, 512], F32, tag="y",
                                          name=f"y{qb}_{g}_{j_}")
                                 for j_ in range(4)]
                        nkc = 4 * (qb + 1)

                        def emit_pv(c, phs, lo, first, last):
                            for pj in range(2):
                                for e in range(2):
                                    j = 2 * pj + e
                                    hloc = 4 * g + j
                                    nc.tensor.matmul(
                                        ybank[j][:, lo:],
                                        vsb[:, c, 65 * hloc:65 * hloc + 65],
                                        phs[pj][:, e, lo:],
                                        start=first, stop=last,
                                        skip_group_check=True,
                                    )

                        # software-pipelined: PV runs two chunks behind the
                        # scores+exp so the PE never waits on the ACT exp.
                        pending = []
                        for c in range(nkc):
                            scs = [psa.tile([128, 2, 512], F32, tag="pa",
                                            name="scA"),
                                   psa.tile([128, 2, 512], F32, tag="pa",
                                            name="scB")]
                            if "F" in sub:
                                # timing-ablation only: WRONG results —
                                # one full-K MM per scs half instead of 8
                                # row-tiled head MMs.
                                for pj in range(2):
                                    for half in range(2):
                                        cc = 2 * g + half
                                        nc.tensor.matmul(
                                            scs[pj][:, pj, :],
                                            khat[:, cc,
                                                 128 * c:128 * (c + 1)],
                                            qhat[:, cc, tqs],
                                            start=(half == 0),
                                            stop=(half == 1),
                                        )
                            else:
                                for j in range(4):
                                    for half in range(2):
                                        cc = 2 * g + half
                                        nc.tensor.matmul(
                                            scs[j // 2][:, j % 2, :],
                                            khat[32 * j:32 * (j + 1), cc,
                                                 128 * c:128 * (c + 1)],
                                            qhat[32 * j:32 * (j + 1), cc,
                                                 tqs],
                                            start=(half == 0),
                                            stop=(half == 1),
                                            tile_position=(32 * j, 0),
                                        )
                            kd = c - 4 * qb
                            # diagonal chunks: only columns >= 128*kd live
                            lo = 128 * kd if kd > 0 else 0
                            phs = []
                            for pj in range(2):
                                ph = tmp.tile([128, 2, 512], BF16, tag="ph",
                                              bufs=6, name=f"ph{pj}")
                                nc.scalar.activation(
                                    ph[:, :, lo:], scs[pj][:, :, lo:],
                                    ACTF.Exp)
                                if kd >= 0 and "M" not in sub:
                                    # diagonal boundary strip
                                    nc.vector.tensor_mul(
                                        ph[:, :, 128 * kd:128 * (kd + 1)],
                                        ph[:, :, 128 * kd:128 * (kd + 1)],
                                        mask_sb[:, None, :].to_broadcast(
                                            [128, 2, 128]),
                                    )
                                phs.append(ph)
                            pending.append((c, phs, lo, c == 0,
                                            c == nkc - 1))
                            if len(pending) > 2:
                                emit_pv(*pending.pop(0))
                        for p in pending:
                            emit_pv(*p)
                        # normalize: yhat rows = y / denom (reciprocal of
                        # the ones-column row, broadcast down 64 rows via
                        # PE). Batched per engine so no stream stalls:
                        # 4 recips (DVE), 2 bcast matmuls of 2 heads each
                        # (PE), 2 copies (ACT), 4 muls (DVE).
                        rcps = []
                        for j in range(4):
                            rcp = tmp.tile([65, 512], BF16, tag="rcp",
                                           bufs=4)
                            with nc.allow_low_precision(
                                    reason="bf16 softmax div"):
                                nc.vector.reciprocal(rcp[64:65, :],
                                                     ybank[j][64:65, :])
                            rcps.append(rcp)
                        rbss = []
                        for pj in range(2):
                            rb = psa.tile([128, 2, 512], F32, tag="pa",
                                          name=f"rb{qb}_{g}_{pj}")
                            for e in range(2):
                                nc.tensor.matmul(
                                    rb[0:64, e, :],
                                    ones_sb[64:65, :],
                                    rcps[2 * pj + e][64:65, :],
                                    start=True, stop=True,
                                    tile_position=(64, 0),
                                    skip_group_check=True,
                                )
                            rbs = tmp.tile([64, 2, 512], BF16, tag="rbs",
                                           bufs=2)
                            nc.scalar.activation(rbs, rb[0:64, :, :],
                                                 ACTF.Copy)
                            rbss.append(rbs)
                        for j in range(4):
                            hloc = 4 * g + j
                            nc.vector.tensor_mul(
                                yhat[64 * (hloc % 2):64 * (hloc % 2 + 1),
                                     hloc // 2, tqs],
                                ybank[j][0:64, :],
                                rbss[j // 2][:, j % 2, :],
                            )

                    # ==== transposed out-projection for this tq block ====
                    # OUT^T[n, t] = sum_m Wo[m, n]^T yhat[m, t]
                    for n8 in range(NO8 if "C" in phases else 0):
                        op = psy.tile([128, 512], F32, tag="y",
                                      name=f"op{n8}_{qb}")
                        for m in range(4):
                            nc.tensor.matmul(
                                op,
                                wo_sb[:, m, 128 * n8:128 * (n8 + 1)],
                                yhat[:, m, tqs],
                                start=(m == 0), stop=(m == 3),
                            )
                        osb = tmp.tile([128, 512], BF16, tag="osb")
                        nc.vector.tensor_copy(out=osb, in_=op)
                        nc.sync.dma_start(
                            out=OUTT[128 * n8:128 * (n8 + 1), tqs],
                            in_=osb)

            # python-unrolled repeat: the Tile For_i back-edge costs ~600us
            # per iteration on this runtime, so unroll instead.
            if "W" in sub:   # timing ablation: weights loaded once
                load_weights()
            for _ in range(repeat):
                body()

    nc.finalize()
    return nc


# ======================================================================
# Full-problem harness: 8 cores = 4 batch x 2 head-groups
# ======================================================================
B_FULL, T_FULL, C_FULL, H_FULL = 4, 2048, 1024, 16

_NC_CACHE = {}


def _get_nc(repeat=1, phases="ABC", sub="qkvrsm"):
    key = ("nc", repeat, phases, sub)
    if key not in _NC_CACHE:
        _NC_CACHE[key] = build_nc(T_FULL, C_FULL, C_FULL, repeat=repeat,
                                  phases=phases, sub=sub)
    return _NC_CACHE[key]


def make_in_maps(x, cos, sin, Wq, Wk, Wv, Wo):
    x, Wq, Wk, Wv, Wo = (np.asarray(a, dtype=np.float32)
                         for a in (x, Wq, Wk, Wv, Wo))
    cos_t = np.asarray(cos, dtype=np.float32)[0, 0]   # [T, 32]
    sin_t = np.asarray(sin, dtype=np.float32)[0, 0]
    consts = make_blob_consts(T_FULL, cos_t, sin_t)
    perm = qk_col_perm()
    KC = C_FULL // 128
    in_maps = []
    for core in range(8):
        b, hg = core // 2, core % 2
        cols = slice(512 * hg, 512 * (hg + 1))
        named = dict(consts)
        named["WQ"] = Wq[:, cols][:, perm].reshape(KC, 128, CH) \
            .transpose(1, 0, 2).reshape(128, KC * CH)
        named["WK"] = Wk[:, cols][:, perm].reshape(KC, 128, CH) \
            .transpose(1, 0, 2).reshape(128, KC * CH)
        named["WV"] = Wv[:, cols].reshape(KC, 128, CH) \
            .transpose(1, 0, 2).reshape(128, KC * CH)
        named["WO"] = Wo[cols, :].reshape(4, 128, C_FULL) \
            .transpose(1, 0, 2).reshape(128, 4 * C_FULL)
        blob = pack_blob(T_FULL, C_FULL, C_FULL, named)
        # [ki, tb, ko, tt] packing: row ki holds, per t-block, all ko
        # chunks' 512-t slices contiguously.
        xt = x[b].T.reshape(KC, 128, T_FULL // 512, 512)
        xt = xt.transpose(1, 2, 0, 3).reshape(128, KC * T_FULL)
        in_maps.append(dict(
            XT=np.ascontiguousarray(xt).astype(BFNP),
            CONST=blob,
        ))
    return in_maps


def gather_out(results):
    out = np.empty((B_FULL, T_FULL, C_FULL), dtype=np.float32)
    for b in range(B_FULL):
        s = (results[2 * b]["OUTT"].astype(np.float32)
             + results[2 * b + 1]["OUTT"].astype(np.float32))
        out[b] = s.T
    return out


def kernel(x, cos, sin, Wq, Wk, Wv, Wo):
    from concourse.bass_utils import run_bass_kernel_spmd
    nc = _get_nc()
    in_maps = make_in_maps(x, cos, sin, Wq, Wk, Wv, Wo)
    res = run_bass_kernel_spmd(nc, in_maps, core_ids=list(range(8)))
    return gather_out(res.results)
